# revision 18
# baseline (speedup 1.0000x reference)
"""Cross-attention Trainium2 kernel (8 NeuronCores), v2.

Sharding: core i handles batch b = i//2 and head-group g = i%2 (8 of 16
heads). Each core computes its partial output projection; the host sums the
two head-group partials per batch and adds the bias (unsharding).

v2 layout: everything stays on-chip (no DRAM staging roundtrips).
  1. x, c, weights stream in as bf16 via converting SWDGE DMAs.
  2. xT/cT produced by TensorE transposes (128x128 tiles -> PSUM) with
     batched PSUM->SBUF drains on DVE.
  3. Q = x@Wq, K = c@Wk natural, RoPE on Pool+DVE, then TensorE transposes
     -> qT/kT (head-dim on partitions). V natural with an appended all-ones
     block column so the AV matmul also emits replicated softmax
     denominators.
  4. Attention per head: per c-block scores matmul (double-buffered PSUM) ->
     exp on ScalarE (scale=1/8 fused, bf16 out, triple-buffered stage) ->
     AV accumulation; reciprocal+multiply normalizes into a_T.
  5. Y_partial = a_T.T @ Wproj_g written straight from PSUM to DRAM by DMA.
"""
import sys

sys.path.insert(0, "/opt/trn_rl_repo")

import numpy as np

import concourse.bass as bass
import concourse.mybir as mybir
from concourse import bacc
from concourse.masks import make_identity
from concourse.tile import TileContext
from concourse.bass_utils import run_bass_kernel_spmd

F32 = mybir.dt.float32
BF16 = mybir.dt.bfloat16
AF = mybir.ActivationFunctionType
ALU = mybir.AluOpType

P = 128
DIM = 1024
H = 8          # heads per core
HD = 64        # head dim
QC = 512       # q/k/v columns per core (H*HD)
NX = 1024      # query tokens
NC = 2048      # context tokens
XB = NX // P   # 8 x token blocks
CB = NC // P   # 16 c token blocks
KC = DIM // P  # 8 contraction chunks for projections
PAIRS = QC // P  # 4 head pairs


def _emit_rope(nc, pool, psum, trig_cos, trig_sin, tb, out_bf):
    """RoPE on a [128, QC] psum tile viewed as [128, H, HD]; writes bf16.

    ScalarE (idle during projections) drains PSUM to SBUF bf16 first so the
    four DVE multiplies run entirely on 2-byte SBUF operands (fast DVE modes).
    """
    qf = pool.tile([P, QC], BF16, name="rope_qf", tag="rope_qf")
    nc.scalar.activation(qf, psum, AF.Copy, scale=1.0)
    qv = qf.rearrange("p (h d) -> p h d", h=H)
    cosb = trig_cos[:, tb, None, :].to_broadcast((P, H, HD))
    sin_lo = trig_sin[:, tb, None, 0:32].to_broadcast((P, H, 32))
    sin_hi = trig_sin[:, tb, None, 32:64].to_broadcast((P, H, 32))
    tmp = pool.tile([P, H, HD], BF16, name="rope_tmp", tag="rope_tmp")
    nc.vector.tensor_tensor(tmp[:, :, 0:32], qv[:, :, 32:64], sin_lo, ALU.mult)
    nc.vector.tensor_tensor(tmp[:, :, 32:64], qv[:, :, 0:32], sin_hi, ALU.mult)
    cq = pool.tile([P, H, HD], BF16, name="rope_cq", tag="rope_cq")
    nc.vector.tensor_tensor(cq, qv, cosb, ALU.mult)
    ov = out_bf.rearrange("p (h d) -> p h d", h=H)
    nc.vector.tensor_tensor(ov, cq, tmp, ALU.add)


def build_kernel():
    nc = bacc.Bacc("TRN2", target_bir_lowering=False, debug=False)

    x_d = nc.dram_tensor("x", [NX, DIM], F32, kind="ExternalInput")
    c_d = nc.dram_tensor("c", [NC, DIM], F32, kind="ExternalInput")
    wq_d = nc.dram_tensor("wq", [DIM, QC], F32, kind="ExternalInput")
    wk_d = nc.dram_tensor("wk", [DIM, QC], F32, kind="ExternalInput")
    wv_d = nc.dram_tensor("wv", [DIM, QC], F32, kind="ExternalInput")
    wp_d = nc.dram_tensor("wp", [QC, DIM], F32, kind="ExternalInput")
    xpos_d = nc.dram_tensor("xpos", [NX, HD], F32, kind="ExternalInput")
    cpos_d = nc.dram_tensor("cpos", [NC, HD], F32, kind="ExternalInput")
    y_d = nc.dram_tensor("y", [NX, DIM], F32, kind="ExternalOutput")

    with TileContext(nc) as tc:
        with tc.tile_pool(name="persist", bufs=1) as pers, \
             tc.tile_pool(name="stage", bufs=2) as stage, \
             tc.tile_pool(name="dram", bufs=1, space="DRAM") as dram:

            # ---------------- constants + input DMAs -----------------------
            ident = pers.tile([P, P], BF16)
            make_identity(nc, ident)

            xpos_sb = pers.tile([P, XB, HD], F32)
            cpos_sb = pers.tile([P, CB, HD], F32)
            nc.sync.dma_start(xpos_sb, xpos_d.rearrange("(o p) d -> p o d", p=P))
            nc.sync.dma_start(cpos_sb, cpos_d.rearrange("(o p) d -> p o d", p=P))

            # v_aug ones block: emitted first so the big memset doesn't queue
            # behind DMA issues on the Pool sequencer
            v_aug = pers.tile([P, CB, H, P], BF16)  # [:, :, :, 0:64]=V, 64:128=ones
            nc.gpsimd.memset(v_aug[:, :, :, HD:P], 1.0)

            # converting loads (f32 DRAM -> bf16) on the SWDGE path. All
            # destinations are dedicated buffers, so no issue ever waits on
            # the sequencer and the explicit order below is the DMA order.
            x_r = x_d.rearrange("(o p) d -> p o d", p=P)
            x_bf = pers.tile([P, XB, DIM], BF16)
            wq_bf = pers.tile([P, KC, QC], BF16)
            wk_bf = pers.tile([P, KC, QC], BF16)
            wv_bf = pers.tile([P, KC, QC], BF16)
            wp_bf = pers.tile([P, PAIRS, DIM], BF16)
            # c: convert f32->bf16 DRAM->DRAM, transposed into SBUF by the
            # DMA crossbar (keeps the TensorEngine free for matmuls)
            cbf_dram = dram.tile([NC, DIM], BF16)
            nc.gpsimd.dma_start(x_bf[:, 0:4, :], x_r[:, 0:4, :])
            nc.gpsimd.dma_start(cbf_dram[0:NX, :], c_d[0:NX, :])
            nc.gpsimd.dma_start(wq_bf, wq_d.rearrange("(o p) n -> p o n", p=P))
            nc.gpsimd.dma_start(x_bf[:, 4:8, :], x_r[:, 4:8, :])
            nc.gpsimd.dma_start(cbf_dram[NX:NC, :], c_d[NX:NC, :])
            nc.gpsimd.dma_start(wk_bf, wk_d.rearrange("(o p) n -> p o n", p=P))
            nc.gpsimd.dma_start(wv_bf, wv_d.rearrange("(o p) n -> p o n", p=P))
            nc.gpsimd.dma_start(wp_bf, wp_d.rearrange("(o p) n -> p o n", p=P))

            # ---------------- trig tables (bf16 values) ---------------------
            cosx = pers.tile([P, XB, HD], BF16)
            sinx = pers.tile([P, XB, HD], BF16)
            cosc = pers.tile([P, CB, HD], BF16)
            sinc = pers.tile([P, CB, HD], BF16)
            # ACT Sin domain is ~[-pi, pi]: wrap args into range first.
            # cos(t) = sin(t + pi/2)
            PI, TWO_PI = float(np.pi), float(2 * np.pi)
            for pos_sb, sin_t, cos_t in ((xpos_sb, sinx, cosx), (cpos_sb, sinc, cosc)):
                rr = stage.tile(list(pos_sb.shape), F32, name="rr", tag="rr", bufs=1)
                nc.vector.add_range_wrap(rr, pos_sb, 0.0, PI, TWO_PI)
                nc.scalar.activation(sin_t, rr, AF.Sin, scale=1.0)
                rr2 = stage.tile(list(pos_sb.shape), F32, name="rr2", tag="rr", bufs=1)
                nc.vector.add_range_wrap(rr2, pos_sb, PI / 2, PI, TWO_PI)
                nc.scalar.activation(cos_t, rr2, AF.Sin, scale=1.0)
                # signed sin: -sin for d<32 (rotation term sign), +sin for d>=32
                nc.vector.tensor_scalar_mul(sin_t[:, :, 0:32], sin_t[:, :, 0:32], -1.0)

            # persistent transposed/derived tensors
            xT = pers.tile([P, KC, NX], BF16)
            cT = pers.tile([P, KC, NC], BF16)
            qT = pers.tile([P, PAIRS, NX], BF16)
            kT = pers.tile([P, PAIRS, NC], BF16)
            a_T = pers.tile([P, PAIRS, NX], BF16)   # normalized out^T

            # ============ phase B/C: transposes + projections ===============
            with tc.tile_pool(name="ps_tr", bufs=2, space="PSUM") as ps_tr, \
                 tc.tile_pool(name="ps_pj", bufs=3, space="PSUM") as ps_pj:

                # x transposes: per token block, 8 feature chunks -> one drain
                for tb in range(XB):
                    ptr = ps_tr.tile([P, KC, P], BF16, name=f"ptx{tb}", tag="ptr8")
                    for ch in range(KC):
                        nc.tensor.transpose(ptr[:, ch, :],
                                            x_bf[:, tb, ch * P:(ch + 1) * P], ident)
                    nc.vector.tensor_copy(xT[:, :, tb * P:(tb + 1) * P], ptr)

                # Q projection (+RoPE) + qT transposes
                for tb in range(XB):
                    pq = ps_pj.tile([P, QC], F32, name=f"pq{tb}", tag="pp")
                    for kc in range(KC):
                        nc.tensor.matmul(pq, xT[:, kc, tb * P:(tb + 1) * P],
                                         wq_bf[:, kc, :],
                                         start=(kc == 0), stop=(kc == KC - 1))
                    q_bf = stage.tile([P, QC], BF16, name=f"q{tb}", tag="q_bf")
                    _emit_rope(nc, stage, pq, cosx, sinx, tb, q_bf)
                    ptq = ps_tr.tile([P, PAIRS, P], BF16, name=f"ptq{tb}", tag="ptr4")
                    for pc in range(PAIRS):
                        nc.tensor.transpose(ptq[:, pc, :],
                                            q_bf[:, pc * P:(pc + 1) * P], ident)
                    nc.vector.tensor_copy(qT[:, :, tb * P:(tb + 1) * P], ptq)

                # c transposes via DMA crossbar (half-c granularity)
                for half in range(2):
                    rs = slice(half * NX, (half + 1) * NX)
                    for ch in range(KC):
                        nc.sync.dma_start_transpose(cT[:, ch, rs],
                                                    cbf_dram[rs, ch * P:(ch + 1) * P])

                # K/V projections + kT transposes + v_aug fill
                for cb in range(CB):
                    pk = ps_pj.tile([P, QC], F32, name=f"pk{cb}", tag="pp")
                    for kc in range(KC):
                        nc.tensor.matmul(pk, cT[:, kc, cb * P:(cb + 1) * P],
                                         wk_bf[:, kc, :],
                                         start=(kc == 0), stop=(kc == KC - 1))
                    k_bf = stage.tile([P, QC], BF16, name=f"k{cb}", tag="q_bf")
                    _emit_rope(nc, stage, pk, cosc, sinc, cb, k_bf)
                    ptk = ps_tr.tile([P, PAIRS, P], BF16, name=f"ptk{cb}", tag="ptr4")
                    for pc in range(PAIRS):
                        nc.tensor.transpose(ptk[:, pc, :],
                                            k_bf[:, pc * P:(pc + 1) * P], ident)
                    nc.vector.tensor_copy(kT[:, :, cb * P:(cb + 1) * P], ptk)

                    pv = ps_pj.tile([P, QC], F32, name=f"pv{cb}", tag="pp")
                    for kc in range(KC):
                        nc.tensor.matmul(pv, cT[:, kc, cb * P:(cb + 1) * P],
                                         wv_bf[:, kc, :],
                                         start=(kc == 0), stop=(kc == KC - 1))
                    nc.scalar.activation(
                        v_aug[:, cb, :, 0:HD],
                        pv.rearrange("p (h d) -> p h d", h=H), AF.Copy, scale=1.0)

            # ============ phase D: attention =================================
            # Schraudolph exp for the DVE-offloaded blocks: bf16 bit pattern of
            # e^(s/8) ~= int16(s * (log2e/8 * 128) + (16256 - C + round_fix))
            SCH_A = float(0.125 * np.log2(np.e) * 128.0)
            SCH_B = float(16256.0 - 5.5 + 0.5)
            I16 = mybir.dt.int16
            with tc.tile_pool(name="ps_s", bufs=2, space="PSUM") as ps_s, \
                 tc.tile_pool(name="ps_av", bufs=2, space="PSUM") as ps_av:

                def emit_scores(h, m):
                    pc, par = h // 2, h % 2
                    rows = slice(par * HD, (par + 1) * HD)
                    s = ps_s.tile([P, NX], F32, name=f"s{h}_{m}", tag="s")
                    for qb in range(2):
                        sl = slice(qb * 512, (qb + 1) * 512)
                        nc.tensor.matmul(s[:, sl],
                                         kT[rows, pc, m * P:(m + 1) * P],
                                         qT[rows, pc, sl],
                                         start=True, stop=True)
                    return s

                def emit_exp(h, m, s):
                    if m % 4 == 1:
                        # DVE Schraudolph exp (int16 bits of bf16)
                        e16 = stage.tile([P, NX], I16, name=f"e16_{h}_{m}",
                                         tag="e16", bufs=2)
                        nc.vector.tensor_scalar(e16, s, SCH_A, SCH_B,
                                                ALU.mult, ALU.add)
                        return e16.bitcast(BF16)
                    e = stage.tile([P, NX], BF16, name=f"e{h}_{m}",
                                   tag="e", bufs=2)
                    nc.scalar.activation(e, s, AF.Exp, scale=0.125)
                    return e

                # software-pipelined: scores(m+1) is emitted before AV(m) so
                # PE's in-order stream never parks behind the exp of block m
                HM = [(h, m) for h in range(H) for m in range(CB)]
                pavs = {}
                s_cur = emit_scores(0, 0)
                e_cur = emit_exp(0, 0, s_cur)
                for idx, (h, m) in enumerate(HM):
                    if m == 0:
                        pavs[h] = ps_av.tile([P, NX], F32, name=f"pav{h}", tag="av")
                    if idx + 1 < len(HM):
                        hn, mn = HM[idx + 1]
                        s_nxt = emit_scores(hn, mn)
                    e_now = e_cur
                    for qb in range(2):
                        sl = slice(qb * 512, (qb + 1) * 512)
                        nc.tensor.matmul(pavs[h][:, sl], v_aug[:, m, h, :],
                                         e_now[:, sl],
                                         start=(m == 0), stop=(m == CB - 1))
                    if idx + 1 < len(HM):
                        e_cur = emit_exp(hn, mn, s_nxt)
                    if m == CB - 1:
                        # rows 64:128 of pav hold the replicated denominator
                        pc, par = h // 2, h % 2
                        rows = slice(par * HD, (par + 1) * HD)
                        recp = stage.tile([P, NX], F32, name=f"rec{h}",
                                          tag="rec", bufs=1)
                        nc.vector.reciprocal(recp[64:128, :], pavs[h][64:128, :])
                        nc.vector.tensor_tensor(a_T[rows, pc, :], pavs[h][0:64, :],
                                                recp[64:128, :], ALU.mult)

            # ============ phase E: output projection =========================
            # 4 psum blocks per y stage tile; one DMA per 2 token blocks
            with tc.tile_pool(name="ps_y", bufs=6, space="PSUM") as ps_y:
                for tb in range(XB):
                    y_sb = stage.tile([P, 2, 512], F32, name=f"y{tb}",
                                      tag="ygrp", bufs=2)
                    for ob in range(2):
                        py = ps_y.tile([P, 512], F32, name=f"py{tb}_{ob}",
                                       tag="py")
                        for kc in range(PAIRS):
                            nc.tensor.matmul(py, a_T[:, kc, tb * P:(tb + 1) * P],
                                             wp_bf[:, kc, ob * 512:(ob + 1) * 512],
                                             start=(kc == 0),
                                             stop=(kc == PAIRS - 1))
                        if ob == 0:
                            nc.vector.tensor_copy(y_sb[:, ob, :], py)
                        else:
                            nc.scalar.activation(y_sb[:, ob, :], py,
                                                 AF.Copy, scale=1.0)
                    nc.sync.dma_start(
                        y_d[tb * P:(tb + 1) * P, :],
                        y_sb.rearrange("p o n -> p (o n)"))
    nc.compile()
    return nc


_NC_CACHE = None


def make_in_maps(inputs):
    x, c = inputs["x"], inputs["c"]
    Wq, Wkv, Wproj = inputs["Wq"], inputs["Wkv"], inputs["Wproj"]
    in_maps = []
    for core in range(8):
        b, g = core // 2, core % 2
        sl = slice(g * QC, (g + 1) * QC)
        in_maps.append(dict(
            x=np.ascontiguousarray(x[b], np.float32),
            c=np.ascontiguousarray(c[b], np.float32),
            wq=np.ascontiguousarray(Wq[:, sl], np.float32),
            wk=np.ascontiguousarray(Wkv[:, sl], np.float32),
            wv=np.ascontiguousarray(Wkv[:, DIM + g * QC: DIM + (g + 1) * QC], np.float32),
            wp=np.ascontiguousarray(Wproj[sl, :], np.float32),
            xpos=np.ascontiguousarray(inputs["x_pos_embed"], np.float32),
            cpos=np.ascontiguousarray(inputs["c_pos_embed"], np.float32),
        ))
    return in_maps


def kernel(x, c, x_pos_embed, c_pos_embed, Wq, Wkv, Wproj, bproj):
    global _NC_CACHE
    if _NC_CACHE is None:
        _NC_CACHE = build_kernel()
    nc = _NC_CACHE

    B = x.shape[0]
    in_maps = make_in_maps(dict(x=x, c=c, Wq=Wq, Wkv=Wkv, Wproj=Wproj,
                                x_pos_embed=x_pos_embed, c_pos_embed=c_pos_embed))

    res = run_bass_kernel_spmd(nc, in_maps, core_ids=list(range(8)))
    out = np.empty((B, NX, DIM), np.float32)
    bias = np.asarray(bproj, np.float32)
    for b in range(B):
        out[b] = res.results[2 * b]["y"] + res.results[2 * b + 1]["y"] + bias
    return out


# revision 21
# speedup vs baseline: 1.0917x; 1.0917x over previous
"""Cross-attention Trainium2 kernel (8 NeuronCores), v2.

Sharding: core i handles batch b = i//2 and head-group g = i%2 (8 of 16
heads). Each core computes its partial output projection; the host sums the
two head-group partials per batch and adds the bias (unsharding).

v2 layout: everything stays on-chip (no DRAM staging roundtrips).
  1. x, c, weights stream in as bf16 via converting SWDGE DMAs.
  2. xT/cT produced by TensorE transposes (128x128 tiles -> PSUM) with
     batched PSUM->SBUF drains on DVE.
  3. Q = x@Wq, K = c@Wk natural, RoPE on Pool+DVE, then TensorE transposes
     -> qT/kT (head-dim on partitions). V natural with an appended all-ones
     block column so the AV matmul also emits replicated softmax
     denominators.
  4. Attention per head: per c-block scores matmul (double-buffered PSUM) ->
     exp on ScalarE (scale=1/8 fused, bf16 out, triple-buffered stage) ->
     AV accumulation; reciprocal+multiply normalizes into a_T.
  5. Y_partial = a_T.T @ Wproj_g written straight from PSUM to DRAM by DMA.
"""
import sys

sys.path.insert(0, "/opt/trn_rl_repo")

import numpy as np

import concourse.bass as bass
import concourse.mybir as mybir
from concourse import bacc
from concourse.masks import make_identity
from concourse.tile import TileContext
from concourse.bass_utils import run_bass_kernel_spmd

F32 = mybir.dt.float32
BF16 = mybir.dt.bfloat16
AF = mybir.ActivationFunctionType
ALU = mybir.AluOpType

P = 128
DIM = 1024
H = 8          # heads per core
HD = 64        # head dim
QC = 512       # q/k/v columns per core (H*HD)
NX = 1024      # query tokens
NC = 2048      # context tokens
XB = NX // P   # 8 x token blocks
CB = NC // P   # 16 c token blocks
KC = DIM // P  # 8 contraction chunks for projections
PAIRS = QC // P  # 4 head pairs


def _emit_rope(nc, pool, psum, trig_cos, trig_sin, tb, out_bf):
    """RoPE on a [128, QC] psum tile viewed as [128, H, HD]; writes bf16.

    ScalarE (idle during projections) drains PSUM into out_bf first so the
    four DVE multiplies run entirely on 2-byte SBUF operands (fast DVE
    modes); the final add overwrites out_bf in place (all ops are DVE,
    in-order, so the WAR hazard is safe).
    """
    nc.scalar.activation(out_bf, psum, AF.Copy, scale=1.0)
    qv = out_bf.rearrange("p (h d) -> p h d", h=H)
    cosb = trig_cos[:, tb, None, :].to_broadcast((P, H, HD))
    sin_lo = trig_sin[:, tb, None, 0:32].to_broadcast((P, H, 32))
    sin_hi = trig_sin[:, tb, None, 32:64].to_broadcast((P, H, 32))
    tmp = pool.tile([P, H, HD], BF16, name="rope_tmp", tag="rope_tmp", bufs=1)
    nc.vector.tensor_tensor(tmp[:, :, 0:32], qv[:, :, 32:64], sin_lo, ALU.mult)
    nc.vector.tensor_tensor(tmp[:, :, 32:64], qv[:, :, 0:32], sin_hi, ALU.mult)
    cq = pool.tile([P, H, HD], BF16, name="rope_cq", tag="rope_cq", bufs=1)
    nc.vector.tensor_tensor(cq, qv, cosb, ALU.mult)
    ov = out_bf.rearrange("p (h d) -> p h d", h=H)
    nc.vector.tensor_tensor(ov, cq, tmp, ALU.add)


def build_kernel():
    nc = bacc.Bacc("TRN2", target_bir_lowering=False, debug=False)

    x_d = nc.dram_tensor("x", [NX, DIM], F32, kind="ExternalInput")
    c_d = nc.dram_tensor("c", [NC, DIM], F32, kind="ExternalInput")
    wq_d = nc.dram_tensor("wq", [DIM, QC], F32, kind="ExternalInput")
    wk_d = nc.dram_tensor("wk", [DIM, QC], F32, kind="ExternalInput")
    wv_d = nc.dram_tensor("wv", [DIM, QC], F32, kind="ExternalInput")
    wp_d = nc.dram_tensor("wp", [QC, DIM], F32, kind="ExternalInput")
    xpos_d = nc.dram_tensor("xpos", [NX, HD], F32, kind="ExternalInput")
    cpos_d = nc.dram_tensor("cpos", [NC, HD], F32, kind="ExternalInput")
    y_d = nc.dram_tensor("y", [NX, DIM], F32, kind="ExternalOutput")

    with TileContext(nc) as tc:
        with tc.tile_pool(name="persist", bufs=1) as pers, \
             tc.tile_pool(name="stage", bufs=2) as stage, \
             tc.tile_pool(name="dram", bufs=1, space="DRAM") as dram:

            # ---------------- constants + input DMAs -----------------------
            ident = pers.tile([P, P], BF16)
            make_identity(nc, ident)

            xpos_sb = pers.tile([P, XB, HD], F32)
            cpos_sb = pers.tile([P, CB, HD], F32)
            nc.sync.dma_start(xpos_sb, xpos_d.rearrange("(o p) d -> p o d", p=P))
            nc.sync.dma_start(cpos_sb, cpos_d.rearrange("(o p) d -> p o d", p=P))

            # v_aug ones block: emitted first so the big memset doesn't queue
            # behind DMA issues on the Pool sequencer
            v_aug = pers.tile([P, CB, H, P], BF16)  # [:, :, :, 0:64]=V, 64:128=ones
            nc.gpsimd.memset(v_aug[:, :, :, HD:P], 1.0)

            # converting loads (f32 DRAM -> bf16) on the SWDGE path. All
            # destinations are dedicated buffers, so no issue ever waits on
            # the sequencer and the explicit order below is the DMA order.
            x_r = x_d.rearrange("(o p) d -> p o d", p=P)
            x_bf = pers.tile([P, XB, DIM], BF16)
            wq_bf = pers.tile([P, KC, QC], BF16)
            wk_bf = pers.tile([P, KC, QC], BF16)
            wv_bf = pers.tile([P, KC, QC], BF16)
            wp_bf = pers.tile([P, PAIRS, DIM], BF16)
            c_r = c_d.rearrange("(o p) d -> p o d", p=P)
            nc.gpsimd.dma_start(x_bf[:, 0:4, :], x_r[:, 0:4, :])
            nc.gpsimd.dma_start(wq_bf, wq_d.rearrange("(o p) n -> p o n", p=P))
            nc.gpsimd.dma_start(x_bf[:, 4:8, :], x_r[:, 4:8, :])
            ce_tiles = []
            for cq in range(8):  # c eighths, 2 token blocks each
                cbt = stage.tile([P, 2, DIM], BF16, name=f"cbf{cq}", tag="cbf")
                nc.gpsimd.dma_start(cbt, c_r[:, 2 * cq:2 * cq + 2, :])
                ce_tiles.append(cbt)
                if cq == 1:
                    nc.gpsimd.dma_start(wk_bf,
                                        wk_d.rearrange("(o p) n -> p o n", p=P))
                if cq == 3:
                    nc.gpsimd.dma_start(wv_bf,
                                        wv_d.rearrange("(o p) n -> p o n", p=P))
                if cq == 5:
                    nc.gpsimd.dma_start(wp_bf,
                                        wp_d.rearrange("(o p) n -> p o n", p=P))

            # ---------------- trig tables (bf16 values) ---------------------
            cosx = pers.tile([P, XB, HD], BF16)
            sinx = pers.tile([P, XB, HD], BF16)
            cosc = pers.tile([P, CB, HD], BF16)
            sinc = pers.tile([P, CB, HD], BF16)
            # ACT Sin domain is ~[-pi, pi]: wrap args into range first.
            # cos(t) = sin(t + pi/2)
            PI, TWO_PI = float(np.pi), float(2 * np.pi)
            chunks = [(xpos_sb, sinx, cosx, 0), (cpos_sb[:, 0:XB], sinc[:, 0:XB], cosc[:, 0:XB], 1),
                      (cpos_sb[:, XB:CB], sinc[:, XB:CB], cosc[:, XB:CB], 2)]
            for pos_sb, sin_t, cos_t, ci in chunks:
                rr = stage.tile([P, XB, HD], F32, name=f"rr{ci}", tag="rr", bufs=1)
                nc.vector.add_range_wrap(rr, pos_sb, 0.0, PI, TWO_PI)
                nc.scalar.activation(sin_t, rr, AF.Sin, scale=1.0)
                rr2 = stage.tile([P, XB, HD], F32, name=f"rr2{ci}", tag="rr", bufs=1)
                nc.vector.add_range_wrap(rr2, pos_sb, PI / 2, PI, TWO_PI)
                nc.scalar.activation(cos_t, rr2, AF.Sin, scale=1.0)
                # signed sin: -sin for d<32 (rotation term sign), +sin for d>=32
                nc.vector.tensor_scalar_mul(sin_t[:, :, 0:32], sin_t[:, :, 0:32], -1.0)

            # persistent transposed/derived tensors
            xT = pers.tile([P, KC, NX], BF16)
            cT = pers.tile([P, KC, NC], BF16)
            qT = pers.tile([P, PAIRS, NX], BF16)
            kT = pers.tile([P, PAIRS, NC], BF16)
            a_T = pers.tile([P, PAIRS, NX], BF16)   # normalized out^T

            # ============ phase B/C: transposes + projections ===============
            with tc.tile_pool(name="ps_tr", bufs=2, space="PSUM") as ps_tr, \
                 tc.tile_pool(name="ps_pj", bufs=3, space="PSUM") as ps_pj:

                # x transposes: per token block, 8 feature chunks -> one drain
                for tb in range(XB):
                    ptr = ps_tr.tile([P, KC, P], BF16, name=f"ptx{tb}", tag="ptr8")
                    for ch in range(KC):
                        nc.tensor.transpose(ptr[:, ch, :],
                                            x_bf[:, tb, ch * P:(ch + 1) * P], ident)
                    nc.vector.tensor_copy(xT[:, :, tb * P:(tb + 1) * P], ptr)

                # Q projection (+RoPE) + qT transposes
                for tb in range(XB):
                    pq = ps_pj.tile([P, QC], F32, name=f"pq{tb}", tag="pp")
                    for kc in range(KC):
                        nc.tensor.matmul(pq, xT[:, kc, tb * P:(tb + 1) * P],
                                         wq_bf[:, kc, :],
                                         start=(kc == 0), stop=(kc == KC - 1))
                    q_bf = stage.tile([P, QC], BF16, name=f"q{tb}", tag="q_bf")
                    _emit_rope(nc, stage, pq, cosx, sinx, tb, q_bf)
                    ptq = ps_tr.tile([P, PAIRS, P], BF16, name=f"ptq{tb}", tag="ptr4")
                    for pc in range(PAIRS):
                        nc.tensor.transpose(ptq[:, pc, :],
                                            q_bf[:, pc * P:(pc + 1) * P], ident)
                    nc.vector.tensor_copy(qT[:, :, tb * P:(tb + 1) * P], ptq)

                # c transposes
                for cb in range(CB):
                    cbt = ce_tiles[cb // 2]
                    ptr = ps_tr.tile([P, KC, P], BF16, name=f"ptc{cb}", tag="ptr8")
                    for ch in range(KC):
                        nc.tensor.transpose(ptr[:, ch, :],
                                            cbt[:, cb % 2, ch * P:(ch + 1) * P], ident)
                    nc.vector.tensor_copy(cT[:, :, cb * P:(cb + 1) * P], ptr)

                # K/V projections + kT transposes + v_aug fill
                for cb in range(CB):
                    pk = ps_pj.tile([P, QC], F32, name=f"pk{cb}", tag="pp")
                    for kc in range(KC):
                        nc.tensor.matmul(pk, cT[:, kc, cb * P:(cb + 1) * P],
                                         wk_bf[:, kc, :],
                                         start=(kc == 0), stop=(kc == KC - 1))
                    k_bf = stage.tile([P, QC], BF16, name=f"k{cb}", tag="q_bf")
                    _emit_rope(nc, stage, pk, cosc, sinc, cb, k_bf)
                    ptk = ps_tr.tile([P, PAIRS, P], BF16, name=f"ptk{cb}", tag="ptr4")
                    for pc in range(PAIRS):
                        nc.tensor.transpose(ptk[:, pc, :],
                                            k_bf[:, pc * P:(pc + 1) * P], ident)
                    nc.vector.tensor_copy(kT[:, :, cb * P:(cb + 1) * P], ptk)

                    pv = ps_pj.tile([P, QC], F32, name=f"pv{cb}", tag="pp")
                    for kc in range(KC):
                        nc.tensor.matmul(pv, cT[:, kc, cb * P:(cb + 1) * P],
                                         wv_bf[:, kc, :],
                                         start=(kc == 0), stop=(kc == KC - 1))
                    nc.scalar.activation(
                        v_aug[:, cb, :, 0:HD],
                        pv.rearrange("p (h d) -> p h d", h=H), AF.Copy, scale=1.0)

            # ============ phase D: attention =================================
            # Schraudolph exp for the DVE-offloaded blocks: bf16 bit pattern of
            # e^(s/8) ~= int16(s * (log2e/8 * 128) + (16256 - C + round_fix))
            SCH_A = float(0.125 * np.log2(np.e) * 128.0)
            SCH_B = float(16256.0 - 5.5 + 0.5)
            I16 = mybir.dt.int16
            with tc.tile_pool(name="ps_s", bufs=2, space="PSUM") as ps_s, \
                 tc.tile_pool(name="ps_av", bufs=2, space="PSUM") as ps_av:

                def emit_scores(h, m):
                    pc, par = h // 2, h % 2
                    rows = slice(par * HD, (par + 1) * HD)
                    s = ps_s.tile([P, NX], F32, name=f"s{h}_{m}", tag="s")
                    for qb in range(2):
                        sl = slice(qb * 512, (qb + 1) * 512)
                        nc.tensor.matmul(s[:, sl],
                                         kT[rows, pc, m * P:(m + 1) * P],
                                         qT[rows, pc, sl],
                                         start=True, stop=True)
                    return s

                def emit_exp(h, m, s):
                    if m % 4 == 1:
                        # DVE Schraudolph exp (int16 bits of bf16)
                        e16 = stage.tile([P, NX], I16, name=f"e16_{h}_{m}",
                                         tag="e16", bufs=2)
                        nc.vector.tensor_scalar(e16, s, SCH_A, SCH_B,
                                                ALU.mult, ALU.add)
                        return e16.bitcast(BF16)
                    e = stage.tile([P, NX], BF16, name=f"e{h}_{m}",
                                   tag="e", bufs=2)
                    nc.scalar.activation(e, s, AF.Exp, scale=0.125)
                    return e

                # software-pipelined: scores(m+1) is emitted before AV(m) so
                # PE's in-order stream never parks behind the exp of block m
                HM = [(h, m) for h in range(H) for m in range(CB)]
                pavs = {}
                s_cur = emit_scores(0, 0)
                e_cur = emit_exp(0, 0, s_cur)
                for idx, (h, m) in enumerate(HM):
                    if m == 0:
                        pavs[h] = ps_av.tile([P, NX], F32, name=f"pav{h}", tag="av")
                    if idx + 1 < len(HM):
                        hn, mn = HM[idx + 1]
                        s_nxt = emit_scores(hn, mn)
                    e_now = e_cur
                    for qb in range(2):
                        sl = slice(qb * 512, (qb + 1) * 512)
                        nc.tensor.matmul(pavs[h][:, sl], v_aug[:, m, h, :],
                                         e_now[:, sl],
                                         start=(m == 0), stop=(m == CB - 1))
                    if idx + 1 < len(HM):
                        e_cur = emit_exp(hn, mn, s_nxt)
                    if m == CB - 1:
                        # rows 64:128 of pav hold the replicated denominator
                        pc, par = h // 2, h % 2
                        rows = slice(par * HD, (par + 1) * HD)
                        recp = stage.tile([P, NX], F32, name=f"rec{h}",
                                          tag="rec", bufs=1)
                        nc.vector.reciprocal(recp[64:128, :], pavs[h][64:128, :])
                        nc.vector.tensor_tensor(a_T[rows, pc, :], pavs[h][0:64, :],
                                                recp[64:128, :], ALU.mult)

            # ============ phase E: output projection =========================
            # 4 psum blocks per y stage tile; one DMA per 2 token blocks
            with tc.tile_pool(name="ps_y", bufs=6, space="PSUM") as ps_y:
                for tb in range(XB):
                    y_sb = stage.tile([P, 2, 512], F32, name=f"y{tb}",
                                      tag="ygrp", bufs=2)
                    for ob in range(2):
                        py = ps_y.tile([P, 512], F32, name=f"py{tb}_{ob}",
                                       tag="py")
                        for kc in range(PAIRS):
                            nc.tensor.matmul(py, a_T[:, kc, tb * P:(tb + 1) * P],
                                             wp_bf[:, kc, ob * 512:(ob + 1) * 512],
                                             start=(kc == 0),
                                             stop=(kc == PAIRS - 1))
                        if ob == 0:
                            nc.vector.tensor_copy(y_sb[:, ob, :], py)
                        else:
                            nc.scalar.activation(y_sb[:, ob, :], py,
                                                 AF.Copy, scale=1.0)
                    nc.sync.dma_start(
                        y_d[tb * P:(tb + 1) * P, :],
                        y_sb.rearrange("p o n -> p (o n)"))
    nc.compile()
    return nc


_NC_CACHE = None


def make_in_maps(inputs):
    x, c = inputs["x"], inputs["c"]
    Wq, Wkv, Wproj = inputs["Wq"], inputs["Wkv"], inputs["Wproj"]
    in_maps = []
    for core in range(8):
        b, g = core // 2, core % 2
        sl = slice(g * QC, (g + 1) * QC)
        in_maps.append(dict(
            x=np.ascontiguousarray(x[b], np.float32),
            c=np.ascontiguousarray(c[b], np.float32),
            wq=np.ascontiguousarray(Wq[:, sl], np.float32),
            wk=np.ascontiguousarray(Wkv[:, sl], np.float32),
            wv=np.ascontiguousarray(Wkv[:, DIM + g * QC: DIM + (g + 1) * QC], np.float32),
            wp=np.ascontiguousarray(Wproj[sl, :], np.float32),
            xpos=np.ascontiguousarray(inputs["x_pos_embed"], np.float32),
            cpos=np.ascontiguousarray(inputs["c_pos_embed"], np.float32),
        ))
    return in_maps


def kernel(x, c, x_pos_embed, c_pos_embed, Wq, Wkv, Wproj, bproj):
    global _NC_CACHE
    if _NC_CACHE is None:
        _NC_CACHE = build_kernel()
    nc = _NC_CACHE

    B = x.shape[0]
    in_maps = make_in_maps(dict(x=x, c=c, Wq=Wq, Wkv=Wkv, Wproj=Wproj,
                                x_pos_embed=x_pos_embed, c_pos_embed=c_pos_embed))

    res = run_bass_kernel_spmd(nc, in_maps, core_ids=list(range(8)))
    out = np.empty((B, NX, DIM), np.float32)
    bias = np.asarray(bproj, np.float32)
    for b in range(B):
        out[b] = res.results[2 * b]["y"] + res.results[2 * b + 1]["y"] + bias
    return out


# revision 23
# speedup vs baseline: 1.1957x; 1.0952x over previous
"""Cross-attention Trainium2 kernel (8 NeuronCores), v2.

Sharding: core i handles batch b = i//2 and head-group g = i%2 (8 of 16
heads). Each core computes its partial output projection; the host sums the
two head-group partials per batch and adds the bias (unsharding).

v2 layout: everything stays on-chip (no DRAM staging roundtrips).
  1. x, c, weights stream in as bf16 via converting SWDGE DMAs.
  2. xT/cT produced by TensorE transposes (128x128 tiles -> PSUM) with
     batched PSUM->SBUF drains on DVE.
  3. Q = x@Wq, K = c@Wk natural, RoPE on Pool+DVE, then TensorE transposes
     -> qT/kT (head-dim on partitions). V natural with an appended all-ones
     block column so the AV matmul also emits replicated softmax
     denominators.
  4. Attention per head: per c-block scores matmul (double-buffered PSUM) ->
     exp on ScalarE (scale=1/8 fused, bf16 out, triple-buffered stage) ->
     AV accumulation; reciprocal+multiply normalizes into a_T.
  5. Y_partial = a_T.T @ Wproj_g written straight from PSUM to DRAM by DMA.
"""
import sys

sys.path.insert(0, "/opt/trn_rl_repo")

import numpy as np

import concourse.bass as bass
import concourse.mybir as mybir
from concourse import bacc
from concourse.masks import make_identity
from concourse.tile import TileContext
from concourse.bass_utils import run_bass_kernel_spmd

F32 = mybir.dt.float32
BF16 = mybir.dt.bfloat16
AF = mybir.ActivationFunctionType
ALU = mybir.AluOpType

P = 128
DIM = 1024
H = 8          # heads per core
HD = 64        # head dim
QC = 512       # q/k/v columns per core (H*HD)
NX = 1024      # query tokens
NC = 2048      # context tokens
XB = NX // P   # 8 x token blocks
CB = NC // P   # 16 c token blocks
KC = DIM // P  # 8 contraction chunks for projections
PAIRS = QC // P  # 4 head pairs


def _emit_rope(nc, pool, psum, trig_cos, trig_sin, tb, out_bf):
    """RoPE on a [128, QC] psum tile viewed as [128, H, HD]; writes bf16.

    ScalarE (idle during projections) drains PSUM into out_bf first so the
    four DVE multiplies run entirely on 2-byte SBUF operands (fast DVE
    modes); the final add overwrites out_bf in place (all ops are DVE,
    in-order, so the WAR hazard is safe).
    """
    nc.scalar.activation(out_bf, psum, AF.Copy, scale=1.0)
    qv = out_bf.rearrange("p (h d) -> p h d", h=H)
    cosb = trig_cos[:, tb, None, :].to_broadcast((P, H, HD))
    sin_lo = trig_sin[:, tb, None, 0:32].to_broadcast((P, H, 32))
    sin_hi = trig_sin[:, tb, None, 32:64].to_broadcast((P, H, 32))
    tmp = pool.tile([P, H, HD], BF16, name="rope_tmp", tag="rope_tmp", bufs=1)
    nc.vector.tensor_tensor(tmp[:, :, 0:32], qv[:, :, 32:64], sin_lo, ALU.mult)
    nc.vector.tensor_tensor(tmp[:, :, 32:64], qv[:, :, 0:32], sin_hi, ALU.mult)
    cq = pool.tile([P, H, HD], BF16, name="rope_cq", tag="rope_cq", bufs=1)
    nc.vector.tensor_tensor(cq, qv, cosb, ALU.mult)
    ov = out_bf.rearrange("p (h d) -> p h d", h=H)
    nc.vector.tensor_tensor(ov, cq, tmp, ALU.add)


def build_kernel():
    nc = bacc.Bacc("TRN2", target_bir_lowering=False, debug=False)

    x_d = nc.dram_tensor("x", [NX, DIM], F32, kind="ExternalInput")
    c_d = nc.dram_tensor("c", [NC, DIM], F32, kind="ExternalInput")
    wq_d = nc.dram_tensor("wq", [DIM, QC], F32, kind="ExternalInput")
    wk_d = nc.dram_tensor("wk", [DIM, QC], F32, kind="ExternalInput")
    wv_d = nc.dram_tensor("wv", [DIM, QC], F32, kind="ExternalInput")
    wp_d = nc.dram_tensor("wp", [QC, DIM], F32, kind="ExternalInput")
    xpos_d = nc.dram_tensor("xpos", [NX, HD], F32, kind="ExternalInput")
    cpos_d = nc.dram_tensor("cpos", [NC, HD], F32, kind="ExternalInput")
    y_d = nc.dram_tensor("y", [NX, DIM], F32, kind="ExternalOutput")

    with TileContext(nc) as tc:
        with tc.tile_pool(name="persist", bufs=1) as pers, \
             tc.tile_pool(name="stage", bufs=2) as stage, \
             tc.tile_pool(name="dram", bufs=1, space="DRAM") as dram:

            # ---------------- constants + input DMAs -----------------------
            ident = pers.tile([P, P], BF16)
            make_identity(nc, ident)

            xpos_sb = pers.tile([P, XB, HD], F32)
            cpos_sb = pers.tile([P, CB, HD], F32)
            nc.sync.dma_start(xpos_sb, xpos_d.rearrange("(o p) d -> p o d", p=P))
            nc.sync.dma_start(cpos_sb, cpos_d.rearrange("(o p) d -> p o d", p=P))

            # v_aug ones block: emitted first so the big memset doesn't queue
            # behind DMA issues on the Pool sequencer
            v_aug = pers.tile([P, CB, H, P], BF16)  # [:, :, :, 0:64]=V, 64:128=ones
            nc.gpsimd.memset(v_aug[:, :, :, HD:P], 1.0)

            # converting loads (f32 DRAM -> bf16) on the SWDGE path. All
            # destinations are dedicated buffers, so no issue ever waits on
            # the sequencer and the explicit order below is the DMA order.
            x_r = x_d.rearrange("(o p) d -> p o d", p=P)
            x_bf = pers.tile([P, XB, DIM], BF16)
            wq_bf = pers.tile([P, KC, QC], BF16)
            wk_bf = pers.tile([P, KC, QC], BF16)
            wv_bf = pers.tile([P, KC, QC], BF16)
            wp_bf = pers.tile([P, PAIRS, DIM], BF16)
            c_r = c_d.rearrange("(o p) d -> p o d", p=P)
            nc.gpsimd.dma_start(x_bf[:, 0:4, :], x_r[:, 0:4, :])
            nc.gpsimd.dma_start(wq_bf, wq_d.rearrange("(o p) n -> p o n", p=P))
            nc.gpsimd.dma_start(x_bf[:, 4:8, :], x_r[:, 4:8, :])
            ce_tiles = []
            for cq in range(8):  # c eighths, 2 token blocks each
                cbt = stage.tile([P, 2, DIM], BF16, name=f"cbf{cq}", tag="cbf")
                nc.gpsimd.dma_start(cbt, c_r[:, 2 * cq:2 * cq + 2, :])
                ce_tiles.append(cbt)
                if cq == 1:
                    nc.gpsimd.dma_start(wk_bf,
                                        wk_d.rearrange("(o p) n -> p o n", p=P))
                if cq == 3:
                    nc.gpsimd.dma_start(wv_bf,
                                        wv_d.rearrange("(o p) n -> p o n", p=P))
                if cq == 5:
                    nc.gpsimd.dma_start(wp_bf,
                                        wp_d.rearrange("(o p) n -> p o n", p=P))

            # ---------------- trig tables (bf16 values) ---------------------
            cosx = pers.tile([P, XB, HD], BF16)
            sinx = pers.tile([P, XB, HD], BF16)
            cosc = pers.tile([P, CB, HD], BF16)
            sinc = pers.tile([P, CB, HD], BF16)
            # ACT Sin domain is ~[-pi, pi]: wrap args into range first.
            # cos(t) = sin(t + pi/2)
            PI, TWO_PI = float(np.pi), float(2 * np.pi)
            chunks = [(xpos_sb, sinx, cosx, 0), (cpos_sb[:, 0:XB], sinc[:, 0:XB], cosc[:, 0:XB], 1),
                      (cpos_sb[:, XB:CB], sinc[:, XB:CB], cosc[:, XB:CB], 2)]
            for pos_sb, sin_t, cos_t, ci in chunks:
                rr = stage.tile([P, XB, HD], F32, name=f"rr{ci}", tag="rr", bufs=1)
                nc.vector.add_range_wrap(rr, pos_sb, 0.0, PI, TWO_PI)
                nc.scalar.activation(sin_t, rr, AF.Sin, scale=1.0)
                rr2 = stage.tile([P, XB, HD], F32, name=f"rr2{ci}", tag="rr", bufs=1)
                nc.vector.add_range_wrap(rr2, pos_sb, PI / 2, PI, TWO_PI)
                nc.scalar.activation(cos_t, rr2, AF.Sin, scale=1.0)
                # signed sin: -sin for d<32 (rotation term sign), +sin for d>=32
                nc.vector.tensor_scalar_mul(sin_t[:, :, 0:32], sin_t[:, :, 0:32], -1.0)

            # persistent transposed/derived tensors
            xT = pers.tile([P, KC, NX], BF16)
            cT = pers.tile([P, KC, NC], BF16)
            qT = pers.tile([P, PAIRS, NX], BF16)
            kT = pers.tile([P, PAIRS, NC], BF16)
            a_T = pers.tile([P, PAIRS, NX], BF16)   # normalized out^T

            # ============ phase B/C: transposes + projections ===============
            with tc.tile_pool(name="ps_tr", bufs=2, space="PSUM") as ps_tr, \
                 tc.tile_pool(name="ps_pj", bufs=3, space="PSUM") as ps_pj:

                # x transposes: per token block, 8 feature chunks -> one drain
                for tb in range(XB):
                    ptr = ps_tr.tile([P, KC, P], BF16, name=f"ptx{tb}", tag="ptr8")
                    for ch in range(KC):
                        nc.tensor.transpose(ptr[:, ch, :],
                                            x_bf[:, tb, ch * P:(ch + 1) * P], ident)
                    nc.vector.tensor_copy(xT[:, :, tb * P:(tb + 1) * P], ptr)

                # Q projection (+RoPE) + qT transposes
                for tb in range(XB):
                    pq = ps_pj.tile([P, QC], F32, name=f"pq{tb}", tag="pp")
                    for kc in range(KC):
                        nc.tensor.matmul(pq, xT[:, kc, tb * P:(tb + 1) * P],
                                         wq_bf[:, kc, :],
                                         start=(kc == 0), stop=(kc == KC - 1))
                    q_bf = stage.tile([P, QC], BF16, name=f"q{tb}", tag="q_bf")
                    _emit_rope(nc, stage, pq, cosx, sinx, tb, q_bf)
                    ptq = ps_tr.tile([P, PAIRS, P], BF16, name=f"ptq{tb}", tag="ptr4")
                    for pc in range(PAIRS):
                        nc.tensor.transpose(ptq[:, pc, :],
                                            q_bf[:, pc * P:(pc + 1) * P], ident)
                    nc.vector.tensor_copy(qT[:, :, tb * P:(tb + 1) * P], ptq)

                # c transposes
                for cb in range(CB):
                    cbt = ce_tiles[cb // 2]
                    ptr = ps_tr.tile([P, KC, P], BF16, name=f"ptc{cb}", tag="ptr8")
                    for ch in range(KC):
                        nc.tensor.transpose(ptr[:, ch, :],
                                            cbt[:, cb % 2, ch * P:(ch + 1) * P], ident)
                    nc.vector.tensor_copy(cT[:, :, cb * P:(cb + 1) * P], ptr)

                # K/V projections + kT transposes + v_aug fill
                for cb in range(CB):
                    pk = ps_pj.tile([P, QC], F32, name=f"pk{cb}", tag="pp")
                    for kc in range(KC):
                        nc.tensor.matmul(pk, cT[:, kc, cb * P:(cb + 1) * P],
                                         wk_bf[:, kc, :],
                                         start=(kc == 0), stop=(kc == KC - 1))
                    k_bf = stage.tile([P, QC], BF16, name=f"k{cb}", tag="q_bf")
                    _emit_rope(nc, stage, pk, cosc, sinc, cb, k_bf)
                    ptk = ps_tr.tile([P, PAIRS, P], BF16, name=f"ptk{cb}", tag="ptr4")
                    for pc in range(PAIRS):
                        nc.tensor.transpose(ptk[:, pc, :],
                                            k_bf[:, pc * P:(pc + 1) * P], ident)
                    nc.vector.tensor_copy(kT[:, :, cb * P:(cb + 1) * P], ptk)

                    pv = ps_pj.tile([P, QC], F32, name=f"pv{cb}", tag="pp")
                    for kc in range(KC):
                        nc.tensor.matmul(pv, cT[:, kc, cb * P:(cb + 1) * P],
                                         wv_bf[:, kc, :],
                                         start=(kc == 0), stop=(kc == KC - 1))
                    nc.scalar.activation(
                        v_aug[:, cb, :, 0:HD],
                        pv.rearrange("p (h d) -> p h d", h=H), AF.Copy, scale=1.0)

            # ============ phase D: attention =================================
            # Schraudolph exp for the DVE-offloaded blocks: bf16 bit pattern of
            # e^(s/8) ~= int16(s * (log2e/8 * 128) + (16256 - C + round_fix))
            SCH_A = float(0.125 * np.log2(np.e) * 128.0)
            SCH_B = float(16256.0 - 5.5 + 0.5)
            I16 = mybir.dt.int16
            with tc.tile_pool(name="ps_s", bufs=3, space="PSUM") as ps_s, \
                 tc.tile_pool(name="ps_av", bufs=1, space="PSUM") as ps_av:

                def emit_scores(h, m):
                    pc, par = h // 2, h % 2
                    rows = slice(par * HD, (par + 1) * HD)
                    s = ps_s.tile([P, NX], F32, name=f"s{h}_{m}", tag="s")
                    for qb in range(2):
                        sl = slice(qb * 512, (qb + 1) * 512)
                        nc.tensor.matmul(s[:, sl],
                                         kT[rows, pc, m * P:(m + 1) * P],
                                         qT[rows, pc, sl],
                                         start=True, stop=True)
                    return s

                def emit_exp(h, m, s):
                    if m % 4 == 1:
                        # DVE Schraudolph exp (int16 bits of bf16)
                        e16 = stage.tile([P, NX], I16, name=f"e16_{h}_{m}",
                                         tag="e16", bufs=2)
                        nc.vector.tensor_scalar(e16, s, SCH_A, SCH_B,
                                                ALU.mult, ALU.add)
                        return e16.bitcast(BF16)
                    e = stage.tile([P, NX], BF16, name=f"e{h}_{m}",
                                   tag="e", bufs=2)
                    nc.scalar.activation(e, s, AF.Exp, scale=0.125)
                    return e

                # software-pipelined, lookahead 2: scores(m+2) and exp(m+1)
                # are both emitted before AV(m), so PE's in-order stream has
                # a full exp-latency of independent work queued ahead of the
                # AV that parks on exp(m)'s semaphore
                HM = [(h, m) for h in range(H) for m in range(CB)]
                pavs = {}
                s_t, e_t = {}, {}
                s_t[0] = emit_scores(*HM[0])
                s_t[1] = emit_scores(*HM[1])
                e_t[0] = emit_exp(*HM[0], s_t[0])
                for idx, (h, m) in enumerate(HM):
                    if idx + 2 < len(HM):
                        hn, mn = HM[idx + 2]
                        s_t[idx + 2] = emit_scores(hn, mn)
                    if idx + 1 < len(HM):
                        hn, mn = HM[idx + 1]
                        e_t[idx + 1] = emit_exp(hn, mn, s_t[idx + 1])
                    if m == 0:
                        pavs[h] = ps_av.tile([P, NX], F32, name=f"pav{h}", tag="av")
                    e_now = e_t.pop(idx)
                    s_t.pop(idx, None)
                    for qb in range(2):
                        sl = slice(qb * 512, (qb + 1) * 512)
                        nc.tensor.matmul(pavs[h][:, sl], v_aug[:, m, h, :],
                                         e_now[:, sl],
                                         start=(m == 0), stop=(m == CB - 1))
                    if m == CB - 1:
                        # rows 64:128 of pav hold the replicated denominator
                        pc, par = h // 2, h % 2
                        rows = slice(par * HD, (par + 1) * HD)
                        recp = stage.tile([P, NX], F32, name=f"rec{h}",
                                          tag="rec", bufs=1)
                        nc.vector.reciprocal(recp[64:128, :], pavs[h][64:128, :])
                        nc.vector.tensor_tensor(a_T[rows, pc, :], pavs[h][0:64, :],
                                                recp[64:128, :], ALU.mult)

            # ============ phase E: output projection =========================
            # 4 psum blocks per y stage tile; one DMA per 2 token blocks
            with tc.tile_pool(name="ps_y", bufs=6, space="PSUM") as ps_y:
                for tb in range(XB):
                    y_sb = stage.tile([P, 2, 512], F32, name=f"y{tb}",
                                      tag="ygrp", bufs=2)
                    for ob in range(2):
                        py = ps_y.tile([P, 512], F32, name=f"py{tb}_{ob}",
                                       tag="py")
                        for kc in range(PAIRS):
                            nc.tensor.matmul(py, a_T[:, kc, tb * P:(tb + 1) * P],
                                             wp_bf[:, kc, ob * 512:(ob + 1) * 512],
                                             start=(kc == 0),
                                             stop=(kc == PAIRS - 1))
                        if ob == 0:
                            nc.vector.tensor_copy(y_sb[:, ob, :], py)
                        else:
                            nc.scalar.activation(y_sb[:, ob, :], py,
                                                 AF.Copy, scale=1.0)
                    nc.sync.dma_start(
                        y_d[tb * P:(tb + 1) * P, :],
                        y_sb.rearrange("p o n -> p (o n)"))
    nc.compile()
    return nc


_NC_CACHE = None


def make_in_maps(inputs):
    x, c = inputs["x"], inputs["c"]
    Wq, Wkv, Wproj = inputs["Wq"], inputs["Wkv"], inputs["Wproj"]
    in_maps = []
    for core in range(8):
        b, g = core // 2, core % 2
        sl = slice(g * QC, (g + 1) * QC)
        in_maps.append(dict(
            x=np.ascontiguousarray(x[b], np.float32),
            c=np.ascontiguousarray(c[b], np.float32),
            wq=np.ascontiguousarray(Wq[:, sl], np.float32),
            wk=np.ascontiguousarray(Wkv[:, sl], np.float32),
            wv=np.ascontiguousarray(Wkv[:, DIM + g * QC: DIM + (g + 1) * QC], np.float32),
            wp=np.ascontiguousarray(Wproj[sl, :], np.float32),
            xpos=np.ascontiguousarray(inputs["x_pos_embed"], np.float32),
            cpos=np.ascontiguousarray(inputs["c_pos_embed"], np.float32),
        ))
    return in_maps


def kernel(x, c, x_pos_embed, c_pos_embed, Wq, Wkv, Wproj, bproj):
    global _NC_CACHE
    if _NC_CACHE is None:
        _NC_CACHE = build_kernel()
    nc = _NC_CACHE

    B = x.shape[0]
    in_maps = make_in_maps(dict(x=x, c=c, Wq=Wq, Wkv=Wkv, Wproj=Wproj,
                                x_pos_embed=x_pos_embed, c_pos_embed=c_pos_embed))

    res = run_bass_kernel_spmd(nc, in_maps, core_ids=list(range(8)))
    out = np.empty((B, NX, DIM), np.float32)
    bias = np.asarray(bproj, np.float32)
    for b in range(B):
        out[b] = res.results[2 * b]["y"] + res.results[2 * b + 1]["y"] + bias
    return out
